# revision 16
# baseline (speedup 1.0000x reference)
"""Trainium2 Bass kernel for CustomSNNLoss — symmetric-cover design.

Rows sorted by (target, batch); layout rotated per core so core c's
window = local key blocks [0, 16) (global blocks [6c-6, 6c+9]) and its
own 768 rows sit at local blocks [6, 12).

Phase W (window): 16 key-tiles x 768 q. PE sim -> ACT exp(scale_b) ->
Sb f8e4 tiles; fp8 DoubleRow one-hot matmuls (combo slots; padded to
the full 128 PE columns per the dual-fp8 ISA restriction) accumulate
batch-combo segment sums in PSUM, evacuated at the phase boundary.

Phase A (far field): symmetric cover — each unordered far block-pair
exp'd ONCE.  Row-orientation fills [128, <=1024]: PE sim -> ACT
exp(scale_t) f8 out + f32 accum_out (own-row partial rowsums).
Non-DR f8 ones-matmul colsums over the exp'd tiles give the mirror
rows' partials (summed on host).  St = Sb^2 squares (DVE, deferred)
feed the class-slot DoubleRow acc_t matmuls here.  Cover (unit = 2
blocks, distances d in units, p = unit index mod 3):
  row-covered: d in [5,12] + extras A(0)={20}, A(1)={4}, A(2)={3,4}
  colsums:     d in [5,11], skipped when d < 4 + ((p+d) mod 3)
Verified exact (every far ordered pair once, window pairs zero).

Host: O(N) epilogue in fp64 replicating reference math, with exact
f8e4 replication of the diagonal terms.
"""

import numpy as np

N, D = 6144, 128
P = 128
NCORES = 8
R = N // NCORES          # 768 rows per core
WKT = 16                 # window key tiles (local blocks 0..15)
MSLOT = 8                # target-class slots (last = ones)
BSLOT = 40               # combo slots
WPAD = 128               # DoubleRow weights padded to full PE width
NT, NB = 20, 5
MIN_T, MAX_T = 0.1, 1.0
TEMP_BATCH = 0.5
EPS = 1e-8

XCOLS = 4864             # packed xnt cols: local [0,4608) + [5888,6144)
NFILL = 18               # A-phase fills per core (6 blocks x 3)

# Per-unit A-phase fill plan: (mm col ranges in packed xnt, width, exp off)
A_FILLS = {
    0: [([(2048, 2560), (2560, 3072)], 1024, 0),
        ([(3072, 3584), (3584, 4096)], 1024, 1024),
        ([(4608, 4864)], 256, 2048)],
    1: [([(2304, 2816), (2816, 3328)], 1024, 0),
        ([(3328, 3840), (3840, 4352)], 1024, 1024)],
    2: [([(2560, 3072), (3072, 3584)], 1024, 0),
        ([(3584, 4096), (4096, 4608)], 1024, 1024)],
}
FILL_BASE = [0, 3, 6, 8, 10, 12]   # rs column base per (2p+b) block
# DoubleRow colsum chunks per unit: exp-buffer offsets (256-key, pair-summed)
A_CO = {
    0: [256 + 256 * j for j in range(6)] + [2048],
    1: [256 * j for j in range(7)],
    2: [256 * j for j in range(7)],
}
# wave split: chunks needing only f0 exps vs later fills
A_CO_W1 = {0: 3, 1: 4, 2: 4}       # first N chunks ready after f0 exps
# exp offset -> local (unpacked) xnt col base per unit
CO_COLBASE = {0: 2048, 1: 2304, 2: 2560}
CO_GENBASE = {0: 0, 1: 4, 2: 8}    # colsum psum generation base per unit
NGEN = 12

_compile_cache = {}
LAST_RESULT = None


def _build(scale_t: float, scale_b: float):
    from contextlib import ExitStack

    import concourse.bacc as bacc
    import concourse.mybir as mybir
    import concourse.tile as tile

    f32 = mybir.dt.float32
    bf16 = mybir.dt.bfloat16
    f8 = mybir.dt.float8e4
    EXP = mybir.ActivationFunctionType.Exp
    DR = mybir.MatmulPerfMode.DoubleRow

    nc = bacc.Bacc("TRN2", target_bir_lowering=False, debug=False,
                   enable_asserts=False)

    xnt = nc.dram_tensor("xnt", [P, XCOLS], bf16, kind="ExternalInput").ap()
    wt = nc.dram_tensor("wt", [P, WKT * WPAD], f8, kind="ExternalInput").ap()
    wb = nc.dram_tensor("wb", [P, WKT * WPAD], f8, kind="ExternalInput").ap()
    on2 = nc.dram_tensor("on2", [P, 256], f8, kind="ExternalInput").ap()
    rs_out = nc.dram_tensor("rs_out", [P, NFILL], f32,
                            kind="ExternalOutput").ap()
    acc_out = nc.dram_tensor("acc_out", [MSLOT + BSLOT, R], f32,
                             kind="ExternalOutput").ap()
    co_out = nc.dram_tensor("co_out", [NGEN, 512], f32,
                            kind="ExternalOutput").ap()
    red_out = nc.dram_tensor("red_out", [P, 4], f32,
                             kind="ExternalOutput").ap()

    with tile.TileContext(nc) as tc, ExitStack() as ctx:
        const = ctx.enter_context(tc.tile_pool(name="const", bufs=1))

        xnt_sb = const.tile([P, XCOLS], bf16, name="xnt_sb")
        wt_sb = const.tile([P, WKT // 2, 2, WPAD], f8, name="wt_sb")
        wb_sb = const.tile([P, WKT // 2, 2, WPAD], f8, name="wb_sb")
        on_sb = const.tile([P, 2, 128], f8, name="on_sb")
        sbf = const.tile([P, WKT, R], f8, name="sbf")
        stf = const.tile([P, WKT, R], f8, name="stf")
        rs_sb = const.tile([P, NFILL], f32, name="rs_sb")
        accb_sb = const.tile([P, R], f32, name="accb_sb")
        acct_sb = const.tile([P, R], f32, name="acct_sb")
        co_sb = const.tile([P, NGEN, 512], f32, name="co_sb")
        red_sb = const.tile([P, 4], f32, name="red_sb")
        warm = const.tile([P, 2], f32, name="warm")

        nc.vector.memset(warm[:], 0.0)
        nc.scalar.activation(warm[:, 1:2], warm[:, 0:1], EXP, scale=1.0)

        # window q columns first, then window keys, weights, far keys
        for (a, b) in ((768, 1024), (1024, 1280), (1280, 1536), (0, 128),
                       (128, 384), (384, 640), (640, 896), (896, 1152),
                       (1152, 1408), (1408, 1664), (1664, 2048)):
            nc.sync.dma_start(xnt_sb[:, a:b], xnt[:, a:b])
        nc.sync.dma_start(wb_sb[:], wb)
        nc.sync.dma_start(on_sb[:], on2)
        nc.sync.dma_start(wt_sb[:], wt)
        for j in range(2048, XCOLS, 512):
            je = min(j + 512, XCOLS)
            nc.sync.dma_start(xnt_sb[:, j:je], xnt[:, j:je])

        # ---- Phase W: window tiles (squares for kt>=12 deferred to A) ----
        psfill = ctx.enter_context(tc.tile_pool(name="psfill", bufs=2,
                                                space="PSUM"))
        with tc.tile_pool(name="paccb", bufs=1, space="PSUM") as paccb:
            accB = [paccb.tile([P, 512], f32, tag=f"ab{p}", name=f"ab{p}")
                    for p in range(3)]

            def accb_mms(kp):
                for p in range(3):
                    nc.tensor.matmul(
                        accB[p][:, 0:256],
                        wb_sb[:, kp, :, :],
                        sbf[:, 2 * kp:2 * kp + 2, 256 * p:256 * p + 256],
                        start=(kp == 0), stop=(kp == WKT // 2 - 1),
                        perf_mode=DR)

            for kt in range(WKT):
                ps = psfill.tile([P, 1024], f32, tag="ps", name="ps")
                ksl = slice(kt * P, (kt + 1) * P)
                nc.tensor.matmul(ps[:, 0:512], xnt_sb[:, ksl],
                                 xnt_sb[:, 768:1280], start=True, stop=True)
                nc.tensor.matmul(ps[:, 512:768], xnt_sb[:, ksl],
                                 xnt_sb[:, 1280:1536], start=True, stop=True)
                nc.scalar.activation(sbf[:, kt, :], ps[:, 0:R], EXP,
                                     scale=scale_b)
                if kt < 12:
                    nc.vector.tensor_mul(stf[:, kt, :], sbf[:, kt, :],
                                         sbf[:, kt, :])
                if 2 <= kt <= 5:
                    # window-mirror partial rowsums for neighbours' extras
                    nc.vector.tensor_reduce(
                        red_sb[:, kt - 2:kt - 1], stf[:, kt, 512:768],
                        mybir.AxisListType.X, mybir.AluOpType.add)
                if kt >= 3 and kt % 2 == 1 and kt < WKT - 1:
                    accb_mms((kt - 3) // 2)
            accb_mms(WKT // 2 - 2)
            accb_mms(WKT // 2 - 1)
            # evacuate combo accumulators so phase A can reuse the banks
            for p in range(3):
                nc.vector.tensor_copy(accb_sb[:, 256 * p:256 * p + 256],
                                      accB[p][:, 0:256])
        nc.sync.dma_start(acc_out[MSLOT:MSLOT + BSLOT, :],
                          accb_sb[0:BSLOT, :])
        nc.sync.dma_start(red_out, red_sb[:])

        # ---- Phase A: far field ----
        # acc_t issue schedule: after (unit, block) -> list of kps
        acct_sched = {(0, 0): [0, 1, 2], (0, 1): [3, 4, 5],
                      (1, 0): [6], (1, 1): [7]}
        # deferred squares: after (unit, block) -> list of kts
        sq_sched = {(0, 0): [12, 13], (0, 1): [14], (1, 0): [15]}
        with tc.tile_pool(name="pco", bufs=1, space="PSUM") as pco, \
                tc.tile_pool(name="pacct", bufs=1, space="PSUM") as pacct, \
                tc.tile_pool(name="pexp", bufs=2) as pexp:
            accT = [pacct.tile([P, 512], f32, tag=f"at{p}", name=f"at{p}")
                    for p in range(3)]

            def acct_mms(kp):
                for p in range(3):
                    nc.tensor.matmul(
                        accT[p][:, 0:256],
                        wt_sb[:, kp, :, :],
                        stf[:, 2 * kp:2 * kp + 2, 256 * p:256 * p + 256],
                        start=(kp == 0), stop=(kp == WKT // 2 - 1),
                        perf_mode=DR)

            exp_tiles = {}
            co_state = {}

            def co_evac(p, g, on_act):
                if on_act:
                    nc.scalar.copy(co_sb[:, g, :], co_state[p][0][:])
                else:
                    nc.vector.tensor_copy(co_sb[:, g, :], co_state[p][0][:])
                nc.sync.dma_start(co_out[g:g + 1, :], co_sb[0:1, g, :])

            def co_chunks(p, k1, on_act=False):
                cp, k = co_state.get(p, (None, 0))
                co_state[p] = [cp, k]
                while k < k1:
                    if k % 2 == 0:
                        if cp is not None:
                            co_evac(p, CO_GENBASE[p] + k // 2 - 1, on_act)
                        cp = pco.tile([P, 512], f32, tag="co", name="co")
                        co_state[p][0] = cp
                    fo = 256 * (k % 2)
                    nc.tensor.matmul(cp[:, fo:fo + 256], on_sb[:, :, :],
                                     exp_tiles[p][:, 0:2,
                                                  A_CO[p][k]:A_CO[p][k] + 256],
                                     start=True, stop=True, perf_mode=DR)
                    k += 1
                    co_state[p][1] = k

            def co_finish(p, on_act=False):
                k = co_state[p][1]
                co_evac(p, CO_GENBASE[p] + (k - 1) // 2, on_act)

            def fill(p, b, fi):
                qcol = 768 + (2 * p + b) * P
                mms, width, eoff = A_FILLS[p][fi]
                ps = psfill.tile([P, 1024], f32, tag="ps", name="ps")
                o = 0
                for (c0, c1) in mms:
                    nc.tensor.matmul(ps[:, o:o + (c1 - c0)],
                                     xnt_sb[:, qcol:qcol + P],
                                     xnt_sb[:, c0:c1],
                                     start=True, stop=True)
                    o += c1 - c0
                gi = FILL_BASE[2 * p + b] + fi
                nc.scalar.activation(
                    exp_tiles[p][:, b, eoff:eoff + width],
                    ps[:, 0:width], EXP, scale=scale_t,
                    accum_out=rs_sb[:, gi:gi + 1])

            for p in range(3):
                exp_tiles[p] = pexp.tile([P, 2, 2304], f8, tag="expu",
                                         name=f"expu{p}")
                nf = len(A_FILLS[p])
                for b in range(2):
                    for fi in range(nf):
                        fill(p, b, fi)
                        if p > 0 and b == 0 and fi == 0:
                            # previous unit's remaining colsums
                            co_chunks(p - 1, len(A_CO[p - 1]))
                            co_finish(p - 1)
                        if b == 1 and fi == 1:
                            co_chunks(p, A_CO_W1[p])
                    for kt in sq_sched.get((p, b), []):
                        nc.vector.tensor_mul(stf[:, kt, :], sbf[:, kt, :],
                                             sbf[:, kt, :])
                    for kp in acct_sched.get((p, b), []):
                        acct_mms(kp)
            co_chunks(2, len(A_CO[2]), on_act=True)
            co_finish(2)

            for p in range(3):
                if p == 1:
                    nc.vector.tensor_copy(acct_sb[:, 256 * p:256 * p + 256],
                                          accT[p][:, 0:256])
                else:
                    nc.scalar.copy(acct_sb[:, 256 * p:256 * p + 256],
                                   accT[p][:, 0:256])

        # ---- write outputs ----
        nc.sync.dma_start(acc_out[0:MSLOT, :], acct_sb[0:MSLOT, :])
        nc.sync.dma_start(rs_out, rs_sb[:])

    nc.compile()
    return nc


def _get_compiled(scale_t, scale_b):
    key = (round(scale_t, 9), round(scale_b, 9))
    if key not in _compile_cache:
        _compile_cache[key] = _build(scale_t, scale_b)
    return _compile_cache[key]


def _bf16(x):
    b = np.ascontiguousarray(x, dtype=np.float32).view(np.uint32)
    r = ((b.astype(np.uint64) + 0x7FFF + ((b >> 16) & 1)) >> 16 << 16)
    return r.astype(np.uint32).view(np.float32)


def _f8(x):
    import ml_dtypes
    return np.asarray(x, np.float32).astype(
        ml_dtypes.float8_e4m3fn).astype(np.float64)


def _numpy_reference(xn, scale_t, scale_b, tg, bt, wt_w, wb_w):
    """Exact host fallback for label distributions the device layout
    cannot handle (never taken for typical inputs)."""
    f = np.float64
    sim = xn.astype(f) @ xn.astype(f).T
    same_t = tg[:, None] == tg[None, :]
    S_t = np.exp(scale_t * sim)
    diag = np.diagonal(S_t)
    pos = (S_t * same_t).sum(1) - diag
    neg = (S_t * ~same_t).sum(1)
    cnt_pos = same_t.sum(1)
    cnt_neg = (~same_t).sum(1)
    valid = (cnt_pos >= 2) & (cnt_neg >= 1)
    pos_s = np.where(valid, pos, 1.0)
    neg_s = np.where(valid, neg, 1.0)
    loss_i = -np.log(pos_s / (pos_s + neg_s))
    lsum = np.bincount(tg, weights=np.where(valid, loss_i, 0.0),
                       minlength=NT)
    vcnt = np.bincount(tg, weights=valid.astype(f), minlength=NT)
    mean = lsum / np.maximum(vcnt, 1.0)
    lt = np.where(vcnt > 0, mean * np.asarray(wt_w, f), 0.0).sum()
    S_b = np.exp(scale_b * sim)
    same_b = bt[:, None] == bt[None, :]
    pm = same_t & same_b
    nm = same_t & ~same_b
    diag_b = np.diagonal(S_b)
    pos_b = (S_b * pm).sum(1) - diag_b
    neg_b = (S_b * nm).sum(1)
    cpb = pm.sum(1)
    cnb = nm.sum(1)
    valid_b = (cpb >= 2) & (cnb >= 1)
    pos_bs = np.where(valid_b, pos_b, 1.0)
    neg_bs = np.where(valid_b, neg_b, 1.0)
    loss_bi = -np.log(pos_bs / (pos_bs + neg_bs))
    inv = np.where(valid_b, 1.0 / np.where(valid_b, loss_bi, 1.0), 0.0)
    lsum_b = np.bincount(bt, weights=inv, minlength=NB)
    vcnt_b = np.bincount(bt, weights=valid_b.astype(f), minlength=NB)
    mean_b = lsum_b / np.maximum(vcnt_b, 1.0)
    lb = np.where(vcnt_b > 0, mean_b * np.asarray(wb_w, f), 0.0).sum()
    return np.float32(0.9 * lt + 0.1 * lb)


def _run_with_retry(nc, in_maps, core_ids, attempts=3):
    import time as _time

    from concourse.bass_utils import run_bass_kernel_spmd

    for i in range(attempts):
        try:
            return run_bass_kernel_spmd(nc, in_maps, core_ids)
        except Exception:
            if i == attempts - 1:
                raise
            _time.sleep(90)


def kernel(input, temperature, weight_target, weight_batch0, targets, batch0):
    global LAST_RESULT
    import ml_dtypes
    BF = ml_dtypes.bfloat16
    F8 = ml_dtypes.float8_e4m3fn

    x = np.asarray(input, dtype=np.float32)
    t = float(np.clip(np.float32(temperature), MIN_T, MAX_T))
    scale_t, scale_b = 1.0 / t, 1.0 / TEMP_BATCH

    norms = np.sqrt((x * x).sum(axis=1, keepdims=True, dtype=np.float32))
    norms = np.maximum(norms, np.float32(EPS))
    xn = _bf16((x / norms).astype(np.float32))
    tg = np.asarray(targets).astype(np.int64)
    bt = np.asarray(batch0).astype(np.int64)
    combo = tg * NB + bt

    order = np.argsort(combo, kind="stable")
    xs = np.ascontiguousarray(xn[order])
    tgs, bts, cbs = tg[order], bt[order], combo[order]
    s_ii = (xs * xs).sum(axis=1, dtype=np.float32)

    tg_change = np.r_[True, tgs[1:] != tgs[:-1]]
    starts = np.where(tg_change)[0]
    run_id = np.cumsum(tg_change) - 1
    run_ends = np.r_[starts[1:], N]
    cls_start = starts[run_id]
    cls_end = run_ends[run_id]

    # square trick requires scale_t == 2*scale_b
    feasible = abs(scale_t - 2.0 * scale_b) < 1e-9
    slot_t = []           # per-core dict class -> slot
    slot_b = []           # per-core dict combo -> slot
    if feasible:
        for c in range(NCORES):
            # window covers global cols [768c-768, 768c+1280) circularly
            if cls_start[c * R] < 768 * c - 768 or \
                    cls_end[(c + 1) * R - 1] > 768 * c + 1280:
                feasible = False
                break
            rows = slice(c * R, (c + 1) * R)
            ucls = np.unique(tgs[rows])
            if len(ucls) > MSLOT - 1:
                feasible = False
                break
            slot_t.append({int(u): i for i, u in enumerate(ucls)})
            slot_b.append({int(u) * NB + b: i * NB + b
                           for i, u in enumerate(ucls) for b in range(NB)})
    if not feasible:
        return _numpy_reference(xn, scale_t, scale_b, tg, bt,
                                weight_target, weight_batch0)

    xsT = xs.T
    packed_cols = np.r_[0:4608, 5888:6144]
    in_maps = []
    for c in range(NCORES):
        rot = (np.arange(N) + 768 * c - 768) % N
        xnt_full = xsT[:, rot]
        xnt_c = np.ascontiguousarray(xnt_full[:, packed_cols]).astype(BF)
        wkeys = rot[:WKT * P]
        ktg, kcb = tgs[wkeys], cbs[wkeys]
        wt_c = np.zeros((WKT, P, WPAD), np.float32)
        wb_c = np.zeros((WKT, P, WPAD), np.float32)
        wt_c[:, :, MSLOT - 1] = 1.0                      # ones slot
        for cls, sl in slot_t[c].items():
            m = (ktg == cls).reshape(WKT, P)
            wt_c[:, :, sl][m] = 1.0
        for cmb, sl in slot_b[c].items():
            m = (kcb == cmb).reshape(WKT, P)
            wb_c[:, :, sl][m] = 1.0
        in_maps.append({
            "xnt": xnt_c,
            "wt": np.ascontiguousarray(
                wt_c.transpose(1, 0, 2).reshape(P, WKT * WPAD)).astype(F8),
            "wb": np.ascontiguousarray(
                wb_c.transpose(1, 0, 2).reshape(P, WKT * WPAD)).astype(F8),
            "on2": np.ones((P, 256), np.float32).astype(F8),
        })

    nc = _get_compiled(scale_t, scale_b)
    LAST_RESULT = _run_with_retry(nc, in_maps, list(range(NCORES)))

    f = np.float64
    samet2 = np.empty(N); pos4 = np.empty(N); own2 = np.empty(N)
    rowsum4 = np.zeros(N)
    winsum = np.empty(N)
    for c in range(NCORES):
        res = LAST_RESULT.results[c]
        rs = res["rs_out"].astype(f)          # [128, 14]
        am = res["acc_out"].astype(f)         # [48, 768]
        co = res["co_out"].astype(f)          # [12, 512]
        red = res["red_out"].astype(f)        # [128, 4]
        rows = np.arange(c * R, (c + 1) * R)
        lcls = tgs[rows]
        lcmb = cbs[rows]
        st_map = np.array([slot_t[c].get(int(u), 0) for u in range(NT)])
        sb_map = np.array([slot_b[c].get(int(u), 0)
                           for u in range(NT * NB)])
        qidx = np.arange(R)
        pos4[rows] = am[st_map[lcls], qidx]
        winsum[rows] = am[MSLOT - 1, qidx]
        accB = am[MSLOT:]
        own2[rows] = accB[sb_map[lcmb], qidx]
        sam = np.zeros(R)
        for cls, sl in slot_t[c].items():
            sel = lcls == cls
            csl = [slot_b[c][cls * NB + b] for b in range(NB)]
            sam[sel] = accB[csl][:, qidx[sel]].sum(axis=0)
        samet2[rows] = sam
        # own-row partial rowsums (3,3,2,2,2,2 fills per block)
        for bi in range(6):
            f0 = FILL_BASE[bi]
            f1 = FILL_BASE[bi + 1] if bi < 5 else NFILL
            rowsum4[c * R + bi * P:c * R + (bi + 1) * P] += \
                rs[:, f0:f1].sum(axis=1)
        # colsum contributions land on the *key* rows (mirror coverage)
        for p in range(3):
            for k, off in enumerate(A_CO[p]):
                g = CO_GENBASE[p] + k // 2
                h = k % 2
                col0 = 5888 if (p == 0 and off == 2048) \
                    else CO_COLBASE[p] + off
                idx = (col0 + 768 * c - 768 + np.arange(256)) % N
                rowsum4[idx] += co[g, 256 * h:256 * h + 256]
        # window-mirror reduces: rows of window blocks 2..5 gain the keys
        # of this core's p2 unit
        for i, L in enumerate((2, 3, 4, 5)):
            idx = (L * P + 768 * c - 768 + np.arange(P)) % N
            rowsum4[idx] += red[:, i]
    rowsum4 += winsum

    # diagonal terms with exact f8 replication of the device chain
    e2 = _f8(np.exp(scale_b * s_ii.astype(f)))
    e4 = _f8(e2 * e2)

    cnt_t = np.bincount(tgs, minlength=NT)
    n_tb = np.zeros((NT, NB), dtype=np.int64)
    np.add.at(n_tb, (tgs, bts), 1)

    pos_t = pos4 - e4
    neg_t = rowsum4 - pos4
    cnt_pos = cnt_t[tgs]
    cnt_neg = N - cnt_pos
    valid = (cnt_pos >= 2) & (cnt_neg >= 1)
    pos_s = np.where(valid, pos_t, 1.0)
    neg_s = np.where(valid, neg_t, 1.0)
    loss_i = -np.log(pos_s / (pos_s + neg_s))
    lsum = np.bincount(tgs, weights=np.where(valid, loss_i, 0.0),
                       minlength=NT)
    vcnt = np.bincount(tgs, weights=valid.astype(f), minlength=NT)
    mean = lsum / np.maximum(vcnt, 1.0)
    wt_w = np.asarray(weight_target).astype(f)
    loss_target = np.where(vcnt > 0, mean * wt_w, 0.0).sum()

    pos_b = own2 - e2
    neg_b = samet2 - own2
    cnt_pos_b = n_tb[tgs, bts]
    cnt_neg_b = cnt_t[tgs] - cnt_pos_b
    valid_b = (cnt_pos_b >= 2) & (cnt_neg_b >= 1)
    pos_bs = np.where(valid_b, pos_b, 1.0)
    neg_bs = np.where(valid_b, neg_b, 1.0)
    loss_bi = -np.log(pos_bs / (pos_bs + neg_bs))
    inv = np.where(valid_b, 1.0 / np.where(valid_b, loss_bi, 1.0), 0.0)
    lsum_b = np.bincount(bts, weights=inv, minlength=NB)
    vcnt_b = np.bincount(bts, weights=valid_b.astype(f), minlength=NB)
    mean_b = lsum_b / np.maximum(vcnt_b, 1.0)
    wb_w = np.asarray(weight_batch0).astype(f)
    loss_batch = np.where(vcnt_b > 0, mean_b * wb_w, 0.0).sum()

    return np.float32(0.9 * loss_target + 0.1 * loss_batch)


# revision 17
# speedup vs baseline: 1.1436x; 1.1436x over previous
"""Trainium2 Bass kernel for CustomSNNLoss — symmetric-cover design.

Rows sorted by (target, batch); layout rotated per core so core c's
window = local key blocks [0, 16) (global blocks [6c-6, 6c+9]) and its
own 768 rows sit at local blocks [6, 12).

Phase W (window): 16 key-tiles x 768 q. PE sim -> ACT exp(scale_b) ->
Sb f8e4 tiles; fp8 DoubleRow one-hot matmuls (combo slots; padded to
the full 128 PE columns per the dual-fp8 ISA restriction) accumulate
batch-combo segment sums in PSUM, evacuated at the phase boundary.

Phase A (far field): symmetric cover — each unordered far block-pair
exp'd ONCE.  Row-orientation fills [128, <=1024]: PE sim -> ACT
exp(scale_t) f8 out + f32 accum_out (own-row partial rowsums).
Non-DR f8 ones-matmul colsums over the exp'd tiles give the mirror
rows' partials (summed on host).  St = Sb^2 squares (DVE, deferred)
feed the class-slot DoubleRow acc_t matmuls here.  Cover (unit = 2
blocks, distances d in units, p = unit index mod 3):
  row-covered: d in [5,12] + extras A(0)={20}, A(1)={4}, A(2)={3,4}
  colsums:     d in [5,11], skipped when d < 4 + ((p+d) mod 3)
Verified exact (every far ordered pair once, window pairs zero).

Host: O(N) epilogue in fp64 replicating reference math, with exact
f8e4 replication of the diagonal terms.
"""

import numpy as np

N, D = 6144, 128
P = 128
NCORES = 8
R = N // NCORES          # 768 rows per core
WKT = 16                 # window key tiles (local blocks 0..15)
MSLOT = 8                # target-class slots (last = ones)
BSLOT = 40               # combo slots
WPAD = 128               # DoubleRow weights padded to full PE width
NT, NB = 20, 5
MIN_T, MAX_T = 0.1, 1.0
TEMP_BATCH = 0.5
EPS = 1e-8

XCOLS = 4864             # packed xnt cols: local [0,4608) + [5888,6144)
NFILL = 18               # A-phase fills per core (6 blocks x 3)

# Per-unit A-phase fill plan: (mm col ranges in packed xnt, width, exp off)
A_FILLS = {
    0: [([(2048, 2560), (2560, 3072)], 1024, 0),
        ([(3072, 3584), (3584, 4096)], 1024, 1024),
        ([(4608, 4864)], 256, 2048)],
    1: [([(2304, 2816), (2816, 3328)], 1024, 0),
        ([(3328, 3840), (3840, 4352)], 1024, 1024)],
    2: [([(2560, 3072), (3072, 3584)], 1024, 0),
        ([(3584, 4096), (4096, 4608)], 1024, 1024)],
}
FILL_BASE = [0, 3, 6, 8, 10, 12]   # rs column base per (2p+b) block
# DoubleRow colsum chunks per unit: exp-buffer offsets (256-key, pair-summed)
A_CO = {
    0: [256 + 256 * j for j in range(6)] + [2048],
    1: [256 * j for j in range(7)],
    2: [256 * j for j in range(7)],
}
# wave split: chunks needing only f0 exps vs later fills
A_CO_W1 = {0: 3, 1: 4, 2: 4}       # first N chunks ready after f0 exps
# exp offset -> local (unpacked) xnt col base per unit
CO_COLBASE = {0: 2048, 1: 2304, 2: 2560}
CO_GENBASE = {0: 0, 1: 4, 2: 8}    # colsum psum generation base per unit
NGEN = 12

_compile_cache = {}
LAST_RESULT = None


def _build(scale_t: float, scale_b: float):
    from contextlib import ExitStack

    import concourse.bacc as bacc
    import concourse.mybir as mybir
    import concourse.tile as tile

    f32 = mybir.dt.float32
    bf16 = mybir.dt.bfloat16
    f8 = mybir.dt.float8e4
    EXP = mybir.ActivationFunctionType.Exp
    DR = mybir.MatmulPerfMode.DoubleRow

    nc = bacc.Bacc("TRN2", target_bir_lowering=False, debug=False,
                   enable_asserts=False)

    xnt = nc.dram_tensor("xnt", [P, XCOLS], bf16, kind="ExternalInput").ap()
    wt = nc.dram_tensor("wt", [P, WKT * WPAD], f8, kind="ExternalInput").ap()
    wb = nc.dram_tensor("wb", [P, WKT * WPAD], f8, kind="ExternalInput").ap()
    on2 = nc.dram_tensor("on2", [P, 256], f8, kind="ExternalInput").ap()
    rs_out = nc.dram_tensor("rs_out", [P, NFILL], f32,
                            kind="ExternalOutput").ap()
    acc_out = nc.dram_tensor("acc_out", [MSLOT + BSLOT, R], f32,
                             kind="ExternalOutput").ap()
    co_out = nc.dram_tensor("co_out", [NGEN, 512], f32,
                            kind="ExternalOutput").ap()
    red_out = nc.dram_tensor("red_out", [P, 4], f32,
                             kind="ExternalOutput").ap()

    with tile.TileContext(nc) as tc, ExitStack() as ctx:
        const = ctx.enter_context(tc.tile_pool(name="const", bufs=1))

        xnt_sb = const.tile([P, XCOLS], bf16, name="xnt_sb")
        wt_sb = const.tile([P, WKT // 2, 2, WPAD], f8, name="wt_sb")
        wb_sb = const.tile([P, WKT // 2, 2, WPAD], f8, name="wb_sb")
        on_sb = const.tile([P, 2, 128], f8, name="on_sb")
        sbf = const.tile([P, WKT, R], f8, name="sbf")
        stf = const.tile([P, WKT, R], f8, name="stf")
        rs_sb = const.tile([P, NFILL], f32, name="rs_sb")
        accb_sb = const.tile([P, R], f32, name="accb_sb")
        acct_sb = const.tile([P, R], f32, name="acct_sb")
        co_sb = const.tile([P, NGEN, 512], f32, name="co_sb")
        red_sb = const.tile([P, 4], f32, name="red_sb")
        warm = const.tile([P, 2], f32, name="warm")

        nc.vector.memset(warm[:], 0.0)
        nc.scalar.activation(warm[:, 1:2], warm[:, 0:1], EXP, scale=1.0)

        # window q columns first, then window keys, weights, far keys
        nc.sync.dma_start(xnt_sb[:, 768:1536], xnt[:, 768:1536])
        nc.sync.dma_start(xnt_sb[:, 0:128], xnt[:, 0:128])
        nc.sync.dma_start(xnt_sb[:, 128:768], xnt[:, 128:768])
        nc.sync.dma_start(wb_sb[:], wb)
        nc.sync.dma_start(on_sb[:], on2)
        nc.sync.dma_start(xnt_sb[:, 1536:2048], xnt[:, 1536:2048])
        nc.sync.dma_start(wt_sb[:], wt)
        for j in range(2048, XCOLS, 512):
            je = min(j + 512, XCOLS)
            nc.sync.dma_start(xnt_sb[:, j:je], xnt[:, j:je])

        # ---- Phase W: window tiles (squares for kt>=12 deferred to A) ----
        psfill = ctx.enter_context(tc.tile_pool(name="psfill", bufs=2,
                                                space="PSUM"))
        with tc.tile_pool(name="paccb", bufs=1, space="PSUM") as paccb:
            accB = [paccb.tile([P, 512], f32, tag=f"ab{p}", name=f"ab{p}")
                    for p in range(3)]

            def accb_mms(kp):
                for p in range(3):
                    nc.tensor.matmul(
                        accB[p][:, 0:256],
                        wb_sb[:, kp, :, :],
                        sbf[:, 2 * kp:2 * kp + 2, 256 * p:256 * p + 256],
                        start=(kp == 0), stop=(kp == WKT // 2 - 1),
                        perf_mode=DR)

            for kt in range(WKT):
                ps = psfill.tile([P, 1024], f32, tag="ps", name="ps")
                ksl = slice(kt * P, (kt + 1) * P)
                nc.tensor.matmul(ps[:, 0:512], xnt_sb[:, ksl],
                                 xnt_sb[:, 768:1280], start=True, stop=True)
                nc.tensor.matmul(ps[:, 512:768], xnt_sb[:, ksl],
                                 xnt_sb[:, 1280:1536], start=True, stop=True)
                nc.scalar.activation(sbf[:, kt, :], ps[:, 0:R], EXP,
                                     scale=scale_b)
                if kt < 12:
                    nc.vector.tensor_mul(stf[:, kt, :], sbf[:, kt, :],
                                         sbf[:, kt, :])
                if 2 <= kt <= 5:
                    # window-mirror partial rowsums for neighbours' extras
                    nc.vector.tensor_reduce(
                        red_sb[:, kt - 2:kt - 1], stf[:, kt, 512:768],
                        mybir.AxisListType.X, mybir.AluOpType.add)
                if kt >= 3 and kt % 2 == 1 and kt < WKT - 1:
                    accb_mms((kt - 3) // 2)
            accb_mms(WKT // 2 - 2)
            accb_mms(WKT // 2 - 1)
            # evacuate combo accumulators so phase A can reuse the banks
            for p in range(3):
                nc.vector.tensor_copy(accb_sb[:, 256 * p:256 * p + 256],
                                      accB[p][:, 0:256])
        nc.sync.dma_start(acc_out[MSLOT:MSLOT + BSLOT, :],
                          accb_sb[0:BSLOT, :])
        nc.sync.dma_start(red_out, red_sb[:])

        # ---- Phase A: far field ----
        # acc_t issue schedule: after (unit, block) -> list of kps
        acct_sched = {(0, 0): [0, 1, 2], (0, 1): [3, 4, 5],
                      (1, 0): [6], (1, 1): [7]}
        # deferred squares: after (unit, block) -> list of kts
        sq_sched = {(0, 0): [12, 13], (0, 1): [14], (1, 0): [15]}
        with tc.tile_pool(name="pco", bufs=1, space="PSUM") as pco, \
                tc.tile_pool(name="pacct", bufs=1, space="PSUM") as pacct, \
                tc.tile_pool(name="pexp", bufs=2) as pexp:
            accT = [pacct.tile([P, 512], f32, tag=f"at{p}", name=f"at{p}")
                    for p in range(3)]

            def acct_mms(kp):
                for p in range(3):
                    nc.tensor.matmul(
                        accT[p][:, 0:256],
                        wt_sb[:, kp, :, :],
                        stf[:, 2 * kp:2 * kp + 2, 256 * p:256 * p + 256],
                        start=(kp == 0), stop=(kp == WKT // 2 - 1),
                        perf_mode=DR)

            exp_tiles = {}
            co_state = {}

            def co_evac(p, g, on_act):
                if on_act:
                    nc.scalar.copy(co_sb[:, g, :], co_state[p][0][:])
                else:
                    nc.vector.tensor_copy(co_sb[:, g, :], co_state[p][0][:])
                nc.sync.dma_start(co_out[g:g + 1, :], co_sb[0:1, g, :])

            def co_chunks(p, k1, on_act=False):
                cp, k = co_state.get(p, (None, 0))
                co_state[p] = [cp, k]
                while k < k1:
                    if k % 2 == 0:
                        if cp is not None:
                            co_evac(p, CO_GENBASE[p] + k // 2 - 1, on_act)
                        cp = pco.tile([P, 512], f32, tag="co", name="co")
                        co_state[p][0] = cp
                    fo = 256 * (k % 2)
                    nc.tensor.matmul(cp[:, fo:fo + 256], on_sb[:, :, :],
                                     exp_tiles[p][:, 0:2,
                                                  A_CO[p][k]:A_CO[p][k] + 256],
                                     start=True, stop=True, perf_mode=DR)
                    k += 1
                    co_state[p][1] = k

            def co_finish(p, on_act=False):
                k = co_state[p][1]
                co_evac(p, CO_GENBASE[p] + (k - 1) // 2, on_act)

            def fill(p, b, fi):
                qcol = 768 + (2 * p + b) * P
                mms, width, eoff = A_FILLS[p][fi]
                ps = psfill.tile([P, 1024], f32, tag="ps", name="ps")
                o = 0
                for (c0, c1) in mms:
                    nc.tensor.matmul(ps[:, o:o + (c1 - c0)],
                                     xnt_sb[:, qcol:qcol + P],
                                     xnt_sb[:, c0:c1],
                                     start=True, stop=True)
                    o += c1 - c0
                gi = FILL_BASE[2 * p + b] + fi
                nc.scalar.activation(
                    exp_tiles[p][:, b, eoff:eoff + width],
                    ps[:, 0:width], EXP, scale=scale_t,
                    accum_out=rs_sb[:, gi:gi + 1])

            for p in range(3):
                exp_tiles[p] = pexp.tile([P, 2, 2304], f8, tag="expu",
                                         name=f"expu{p}")
                nf = len(A_FILLS[p])
                for b in range(2):
                    for fi in range(nf):
                        fill(p, b, fi)
                        if p > 0 and b == 0 and fi == 0:
                            # previous unit's remaining colsums
                            co_chunks(p - 1, len(A_CO[p - 1]))
                            co_finish(p - 1)
                        if b == 1 and fi == 1:
                            co_chunks(p, A_CO_W1[p])
                    for kt in sq_sched.get((p, b), []):
                        nc.vector.tensor_mul(stf[:, kt, :], sbf[:, kt, :],
                                             sbf[:, kt, :])
                    for kp in acct_sched.get((p, b), []):
                        acct_mms(kp)
            co_chunks(2, len(A_CO[2]), on_act=True)
            co_finish(2, on_act=True)

            for p in range(3):
                if p == 0:
                    nc.scalar.copy(acct_sb[:, 256 * p:256 * p + 256],
                                   accT[p][:, 0:256])
                else:
                    nc.vector.tensor_copy(acct_sb[:, 256 * p:256 * p + 256],
                                          accT[p][:, 0:256])

        # ---- write outputs ----
        nc.sync.dma_start(acc_out[0:MSLOT, :], acct_sb[0:MSLOT, :])
        nc.sync.dma_start(rs_out, rs_sb[:])

    nc.compile()
    return nc


def _get_compiled(scale_t, scale_b):
    key = (round(scale_t, 9), round(scale_b, 9))
    if key not in _compile_cache:
        _compile_cache[key] = _build(scale_t, scale_b)
    return _compile_cache[key]


def _bf16(x):
    b = np.ascontiguousarray(x, dtype=np.float32).view(np.uint32)
    r = ((b.astype(np.uint64) + 0x7FFF + ((b >> 16) & 1)) >> 16 << 16)
    return r.astype(np.uint32).view(np.float32)


def _f8(x):
    import ml_dtypes
    return np.asarray(x, np.float32).astype(
        ml_dtypes.float8_e4m3fn).astype(np.float64)


def _numpy_reference(xn, scale_t, scale_b, tg, bt, wt_w, wb_w):
    """Exact host fallback for label distributions the device layout
    cannot handle (never taken for typical inputs)."""
    f = np.float64
    sim = xn.astype(f) @ xn.astype(f).T
    same_t = tg[:, None] == tg[None, :]
    S_t = np.exp(scale_t * sim)
    diag = np.diagonal(S_t)
    pos = (S_t * same_t).sum(1) - diag
    neg = (S_t * ~same_t).sum(1)
    cnt_pos = same_t.sum(1)
    cnt_neg = (~same_t).sum(1)
    valid = (cnt_pos >= 2) & (cnt_neg >= 1)
    pos_s = np.where(valid, pos, 1.0)
    neg_s = np.where(valid, neg, 1.0)
    loss_i = -np.log(pos_s / (pos_s + neg_s))
    lsum = np.bincount(tg, weights=np.where(valid, loss_i, 0.0),
                       minlength=NT)
    vcnt = np.bincount(tg, weights=valid.astype(f), minlength=NT)
    mean = lsum / np.maximum(vcnt, 1.0)
    lt = np.where(vcnt > 0, mean * np.asarray(wt_w, f), 0.0).sum()
    S_b = np.exp(scale_b * sim)
    same_b = bt[:, None] == bt[None, :]
    pm = same_t & same_b
    nm = same_t & ~same_b
    diag_b = np.diagonal(S_b)
    pos_b = (S_b * pm).sum(1) - diag_b
    neg_b = (S_b * nm).sum(1)
    cpb = pm.sum(1)
    cnb = nm.sum(1)
    valid_b = (cpb >= 2) & (cnb >= 1)
    pos_bs = np.where(valid_b, pos_b, 1.0)
    neg_bs = np.where(valid_b, neg_b, 1.0)
    loss_bi = -np.log(pos_bs / (pos_bs + neg_bs))
    inv = np.where(valid_b, 1.0 / np.where(valid_b, loss_bi, 1.0), 0.0)
    lsum_b = np.bincount(bt, weights=inv, minlength=NB)
    vcnt_b = np.bincount(bt, weights=valid_b.astype(f), minlength=NB)
    mean_b = lsum_b / np.maximum(vcnt_b, 1.0)
    lb = np.where(vcnt_b > 0, mean_b * np.asarray(wb_w, f), 0.0).sum()
    return np.float32(0.9 * lt + 0.1 * lb)


def _run_with_retry(nc, in_maps, core_ids, attempts=3):
    import time as _time

    from concourse.bass_utils import run_bass_kernel_spmd

    for i in range(attempts):
        try:
            return run_bass_kernel_spmd(nc, in_maps, core_ids)
        except Exception:
            if i == attempts - 1:
                raise
            _time.sleep(90)


def kernel(input, temperature, weight_target, weight_batch0, targets, batch0):
    global LAST_RESULT
    import ml_dtypes
    BF = ml_dtypes.bfloat16
    F8 = ml_dtypes.float8_e4m3fn

    x = np.asarray(input, dtype=np.float32)
    t = float(np.clip(np.float32(temperature), MIN_T, MAX_T))
    scale_t, scale_b = 1.0 / t, 1.0 / TEMP_BATCH

    norms = np.sqrt((x * x).sum(axis=1, keepdims=True, dtype=np.float32))
    norms = np.maximum(norms, np.float32(EPS))
    xn = _bf16((x / norms).astype(np.float32))
    tg = np.asarray(targets).astype(np.int64)
    bt = np.asarray(batch0).astype(np.int64)
    combo = tg * NB + bt

    order = np.argsort(combo, kind="stable")
    xs = np.ascontiguousarray(xn[order])
    tgs, bts, cbs = tg[order], bt[order], combo[order]
    s_ii = (xs * xs).sum(axis=1, dtype=np.float32)

    tg_change = np.r_[True, tgs[1:] != tgs[:-1]]
    starts = np.where(tg_change)[0]
    run_id = np.cumsum(tg_change) - 1
    run_ends = np.r_[starts[1:], N]
    cls_start = starts[run_id]
    cls_end = run_ends[run_id]

    # square trick requires scale_t == 2*scale_b
    feasible = abs(scale_t - 2.0 * scale_b) < 1e-9
    slot_t = []           # per-core dict class -> slot
    slot_b = []           # per-core dict combo -> slot
    if feasible:
        for c in range(NCORES):
            # window covers global cols [768c-768, 768c+1280) circularly
            if cls_start[c * R] < 768 * c - 768 or \
                    cls_end[(c + 1) * R - 1] > 768 * c + 1280:
                feasible = False
                break
            rows = slice(c * R, (c + 1) * R)
            ucls = np.unique(tgs[rows])
            if len(ucls) > MSLOT - 1:
                feasible = False
                break
            slot_t.append({int(u): i for i, u in enumerate(ucls)})
            slot_b.append({int(u) * NB + b: i * NB + b
                           for i, u in enumerate(ucls) for b in range(NB)})
    if not feasible:
        return _numpy_reference(xn, scale_t, scale_b, tg, bt,
                                weight_target, weight_batch0)

    xsT = xs.T
    packed_cols = np.r_[0:4608, 5888:6144]
    in_maps = []
    for c in range(NCORES):
        rot = (np.arange(N) + 768 * c - 768) % N
        xnt_full = xsT[:, rot]
        xnt_c = np.ascontiguousarray(xnt_full[:, packed_cols]).astype(BF)
        wkeys = rot[:WKT * P]
        ktg, kcb = tgs[wkeys], cbs[wkeys]
        wt_c = np.zeros((WKT, P, WPAD), np.float32)
        wb_c = np.zeros((WKT, P, WPAD), np.float32)
        wt_c[:, :, MSLOT - 1] = 1.0                      # ones slot
        for cls, sl in slot_t[c].items():
            m = (ktg == cls).reshape(WKT, P)
            wt_c[:, :, sl][m] = 1.0
        for cmb, sl in slot_b[c].items():
            m = (kcb == cmb).reshape(WKT, P)
            wb_c[:, :, sl][m] = 1.0
        in_maps.append({
            "xnt": xnt_c,
            "wt": np.ascontiguousarray(
                wt_c.transpose(1, 0, 2).reshape(P, WKT * WPAD)).astype(F8),
            "wb": np.ascontiguousarray(
                wb_c.transpose(1, 0, 2).reshape(P, WKT * WPAD)).astype(F8),
            "on2": np.ones((P, 256), np.float32).astype(F8),
        })

    nc = _get_compiled(scale_t, scale_b)
    LAST_RESULT = _run_with_retry(nc, in_maps, list(range(NCORES)))

    f = np.float64
    samet2 = np.empty(N); pos4 = np.empty(N); own2 = np.empty(N)
    rowsum4 = np.zeros(N)
    winsum = np.empty(N)
    for c in range(NCORES):
        res = LAST_RESULT.results[c]
        rs = res["rs_out"].astype(f)          # [128, 14]
        am = res["acc_out"].astype(f)         # [48, 768]
        co = res["co_out"].astype(f)          # [12, 512]
        red = res["red_out"].astype(f)        # [128, 4]
        rows = np.arange(c * R, (c + 1) * R)
        lcls = tgs[rows]
        lcmb = cbs[rows]
        st_map = np.array([slot_t[c].get(int(u), 0) for u in range(NT)])
        sb_map = np.array([slot_b[c].get(int(u), 0)
                           for u in range(NT * NB)])
        qidx = np.arange(R)
        pos4[rows] = am[st_map[lcls], qidx]
        winsum[rows] = am[MSLOT - 1, qidx]
        accB = am[MSLOT:]
        own2[rows] = accB[sb_map[lcmb], qidx]
        sam = np.zeros(R)
        for cls, sl in slot_t[c].items():
            sel = lcls == cls
            csl = [slot_b[c][cls * NB + b] for b in range(NB)]
            sam[sel] = accB[csl][:, qidx[sel]].sum(axis=0)
        samet2[rows] = sam
        # own-row partial rowsums (3,3,2,2,2,2 fills per block)
        for bi in range(6):
            f0 = FILL_BASE[bi]
            f1 = FILL_BASE[bi + 1] if bi < 5 else NFILL
            rowsum4[c * R + bi * P:c * R + (bi + 1) * P] += \
                rs[:, f0:f1].sum(axis=1)
        # colsum contributions land on the *key* rows (mirror coverage)
        for p in range(3):
            for k, off in enumerate(A_CO[p]):
                g = CO_GENBASE[p] + k // 2
                h = k % 2
                col0 = 5888 if (p == 0 and off == 2048) \
                    else CO_COLBASE[p] + off
                idx = (col0 + 768 * c - 768 + np.arange(256)) % N
                rowsum4[idx] += co[g, 256 * h:256 * h + 256]
        # window-mirror reduces: rows of window blocks 2..5 gain the keys
        # of this core's p2 unit
        for i, L in enumerate((2, 3, 4, 5)):
            idx = (L * P + 768 * c - 768 + np.arange(P)) % N
            rowsum4[idx] += red[:, i]
    rowsum4 += winsum

    # diagonal terms with exact f8 replication of the device chain
    e2 = _f8(np.exp(scale_b * s_ii.astype(f)))
    e4 = _f8(e2 * e2)

    cnt_t = np.bincount(tgs, minlength=NT)
    n_tb = np.zeros((NT, NB), dtype=np.int64)
    np.add.at(n_tb, (tgs, bts), 1)

    pos_t = pos4 - e4
    neg_t = rowsum4 - pos4
    cnt_pos = cnt_t[tgs]
    cnt_neg = N - cnt_pos
    valid = (cnt_pos >= 2) & (cnt_neg >= 1)
    pos_s = np.where(valid, pos_t, 1.0)
    neg_s = np.where(valid, neg_t, 1.0)
    loss_i = -np.log(pos_s / (pos_s + neg_s))
    lsum = np.bincount(tgs, weights=np.where(valid, loss_i, 0.0),
                       minlength=NT)
    vcnt = np.bincount(tgs, weights=valid.astype(f), minlength=NT)
    mean = lsum / np.maximum(vcnt, 1.0)
    wt_w = np.asarray(weight_target).astype(f)
    loss_target = np.where(vcnt > 0, mean * wt_w, 0.0).sum()

    pos_b = own2 - e2
    neg_b = samet2 - own2
    cnt_pos_b = n_tb[tgs, bts]
    cnt_neg_b = cnt_t[tgs] - cnt_pos_b
    valid_b = (cnt_pos_b >= 2) & (cnt_neg_b >= 1)
    pos_bs = np.where(valid_b, pos_b, 1.0)
    neg_bs = np.where(valid_b, neg_b, 1.0)
    loss_bi = -np.log(pos_bs / (pos_bs + neg_bs))
    inv = np.where(valid_b, 1.0 / np.where(valid_b, loss_bi, 1.0), 0.0)
    lsum_b = np.bincount(bts, weights=inv, minlength=NB)
    vcnt_b = np.bincount(bts, weights=valid_b.astype(f), minlength=NB)
    mean_b = lsum_b / np.maximum(vcnt_b, 1.0)
    wb_w = np.asarray(weight_batch0).astype(f)
    loss_batch = np.where(vcnt_b > 0, mean_b * wb_w, 0.0).sum()

    return np.float32(0.9 * loss_target + 0.1 * loss_batch)


# revision 18
# speedup vs baseline: 1.1994x; 1.0487x over previous
"""Trainium2 Bass kernel for CustomSNNLoss — symmetric-cover design.

Rows sorted by (target, batch); layout rotated per core so core c's
window = local key blocks [0, 16) (global blocks [6c-6, 6c+9]) and its
own 768 rows sit at local blocks [6, 12).

Phase W (window): 16 key-tiles x 768 q. PE sim -> ACT exp(scale_b) ->
Sb f8e4 tiles; fp8 DoubleRow one-hot matmuls (combo slots; padded to
the full 128 PE columns per the dual-fp8 ISA restriction) accumulate
batch-combo segment sums in PSUM, evacuated at the phase boundary.

Phase A (far field): symmetric cover — each unordered far block-pair
exp'd ONCE.  Row-orientation fills [128, <=1024]: PE sim -> ACT
exp(scale_t) f8 out + f32 accum_out (own-row partial rowsums).
Non-DR f8 ones-matmul colsums over the exp'd tiles give the mirror
rows' partials (summed on host).  St = Sb^2 squares (DVE, deferred)
feed the class-slot DoubleRow acc_t matmuls here.  Cover (unit = 2
blocks, distances d in units, p = unit index mod 3):
  row-covered: d in [5,12] + extras A(0)={20}, A(1)={4}, A(2)={3,4}
  colsums:     d in [5,11], skipped when d < 4 + ((p+d) mod 3)
Verified exact (every far ordered pair once, window pairs zero).

Host: O(N) epilogue in fp64 replicating reference math, with exact
f8e4 replication of the diagonal terms.
"""

import numpy as np

N, D = 6144, 128
P = 128
NCORES = 8
R = N // NCORES          # 768 rows per core
WKT = 16                 # window key tiles (local blocks 0..15)
MSLOT = 8                # target-class slots (last = ones)
BSLOT = 40               # combo slots
WPAD = 128               # DoubleRow weights padded to full PE width
NT, NB = 20, 5
MIN_T, MAX_T = 0.1, 1.0
TEMP_BATCH = 0.5
EPS = 1e-8

XCOLS = 4864             # packed xnt cols: local [0,4608) + [5888,6144)
NFILL = 18               # A-phase fills per core (6 blocks x 3)

# Per-unit A-phase fill plan: (mm col ranges in packed xnt, width, exp off)
A_FILLS = {
    0: [([(2048, 2560), (2560, 3072)], 1024, 0),
        ([(3072, 3584), (3584, 4096)], 1024, 1024),
        ([(4608, 4864)], 256, 2048)],
    1: [([(2304, 2816), (2816, 3328)], 1024, 0),
        ([(3328, 3840), (3840, 4352)], 1024, 1024)],
    2: [([(2560, 3072), (3072, 3584)], 1024, 0),
        ([(3584, 4096), (4096, 4608)], 1024, 1024)],
}
FILL_BASE = [0, 3, 6, 8, 10, 12]   # rs column base per (2p+b) block
# DoubleRow colsum chunks per unit: exp-buffer offsets (256-key, pair-summed)
A_CO = {
    0: [256 + 256 * j for j in range(6)] + [2048],
    1: [256 * j for j in range(7)],
    2: [256 * j for j in range(7)],
}
# chunks ready after fill fi (both blocks): cumulative counts per unit
A_CO_WAVE = {0: [3, 6, 7], 1: [4, 7], 2: [4, 7]}
# exp offset -> local (unpacked) xnt col base per unit
CO_COLBASE = {0: 2048, 1: 2304, 2: 2560}
CO_GENBASE = {0: 0, 1: 4, 2: 8}    # colsum psum generation base per unit
NGEN = 12

_compile_cache = {}
LAST_RESULT = None


def _build(scale_t: float, scale_b: float):
    from contextlib import ExitStack

    import concourse.bacc as bacc
    import concourse.mybir as mybir
    import concourse.tile as tile

    f32 = mybir.dt.float32
    bf16 = mybir.dt.bfloat16
    f8 = mybir.dt.float8e4
    EXP = mybir.ActivationFunctionType.Exp
    DR = mybir.MatmulPerfMode.DoubleRow

    nc = bacc.Bacc("TRN2", target_bir_lowering=False, debug=False,
                   enable_asserts=False)

    xnt = nc.dram_tensor("xnt", [P, XCOLS], bf16, kind="ExternalInput").ap()
    wt = nc.dram_tensor("wt", [P, WKT * WPAD], f8, kind="ExternalInput").ap()
    wb = nc.dram_tensor("wb", [P, WKT * WPAD], f8, kind="ExternalInput").ap()
    on2 = nc.dram_tensor("on2", [P, 256], f8, kind="ExternalInput").ap()
    rs_out = nc.dram_tensor("rs_out", [P, NFILL], f32,
                            kind="ExternalOutput").ap()
    acc_out = nc.dram_tensor("acc_out", [MSLOT + BSLOT, R], f32,
                             kind="ExternalOutput").ap()
    co_out = nc.dram_tensor("co_out", [NGEN, 512], f32,
                            kind="ExternalOutput").ap()
    red_out = nc.dram_tensor("red_out", [P, 4], f32,
                             kind="ExternalOutput").ap()

    with tile.TileContext(nc) as tc, ExitStack() as ctx:
        const = ctx.enter_context(tc.tile_pool(name="const", bufs=1))

        xnt_sb = const.tile([P, XCOLS], bf16, name="xnt_sb")
        wt_sb = const.tile([P, WKT // 2, 2, WPAD], f8, name="wt_sb")
        wb_sb = const.tile([P, WKT // 2, 2, WPAD], f8, name="wb_sb")
        on_sb = const.tile([P, 2, 128], f8, name="on_sb")
        sbf = const.tile([P, WKT, R], f8, name="sbf")
        stf = const.tile([P, WKT, R], f8, name="stf")
        rs_sb = const.tile([P, NFILL], f32, name="rs_sb")
        accb_sb = const.tile([P, R], f32, name="accb_sb")
        acct_sb = const.tile([P, R], f32, name="acct_sb")
        co_sb = const.tile([P, NGEN, 512], f32, name="co_sb")
        red_sb = const.tile([P, 4], f32, name="red_sb")
        warm = const.tile([P, 2], f32, name="warm")

        nc.vector.memset(warm[:], 0.0)
        nc.scalar.activation(warm[:, 1:2], warm[:, 0:1], EXP, scale=1.0)

        # window q columns first, then window keys, weights, far keys
        nc.sync.dma_start(xnt_sb[:, 768:1536], xnt[:, 768:1536])
        nc.sync.dma_start(xnt_sb[:, 0:128], xnt[:, 0:128])
        nc.sync.dma_start(xnt_sb[:, 128:768], xnt[:, 128:768])
        nc.sync.dma_start(wb_sb[:], wb)
        nc.sync.dma_start(on_sb[:], on2)
        nc.sync.dma_start(xnt_sb[:, 1536:2048], xnt[:, 1536:2048])
        nc.sync.dma_start(wt_sb[:], wt)
        for j in range(2048, XCOLS, 512):
            je = min(j + 512, XCOLS)
            nc.sync.dma_start(xnt_sb[:, j:je], xnt[:, j:je])

        # ---- Phase W: window tiles (squares for kt>=12 deferred to A) ----
        psfill = ctx.enter_context(tc.tile_pool(name="psfill", bufs=2,
                                                space="PSUM"))
        with tc.tile_pool(name="paccb", bufs=1, space="PSUM") as paccb:
            accB = [paccb.tile([P, 512], f32, tag=f"ab{p}", name=f"ab{p}")
                    for p in range(3)]

            def accb_mms(kp):
                for p in range(3):
                    nc.tensor.matmul(
                        accB[p][:, 0:256],
                        wb_sb[:, kp, :, :],
                        sbf[:, 2 * kp:2 * kp + 2, 256 * p:256 * p + 256],
                        start=(kp == 0), stop=(kp == WKT // 2 - 1),
                        perf_mode=DR)

            for kt in range(WKT):
                ps = psfill.tile([P, 1024], f32, tag="ps", name="ps")
                ksl = slice(kt * P, (kt + 1) * P)
                nc.tensor.matmul(ps[:, 0:512], xnt_sb[:, ksl],
                                 xnt_sb[:, 768:1280], start=True, stop=True)
                nc.tensor.matmul(ps[:, 512:768], xnt_sb[:, ksl],
                                 xnt_sb[:, 1280:1536], start=True, stop=True)
                nc.scalar.activation(sbf[:, kt, :], ps[:, 0:R], EXP,
                                     scale=scale_b)
                if kt < 12:
                    nc.vector.tensor_mul(stf[:, kt, :], sbf[:, kt, :],
                                         sbf[:, kt, :])
                if 2 <= kt <= 5:
                    # window-mirror partial rowsums for neighbours' extras
                    nc.vector.tensor_reduce(
                        red_sb[:, kt - 2:kt - 1], stf[:, kt, 512:768],
                        mybir.AxisListType.X, mybir.AluOpType.add)
                if kt >= 3 and kt % 2 == 1 and kt < WKT - 1:
                    accb_mms((kt - 3) // 2)
            accb_mms(WKT // 2 - 2)
            accb_mms(WKT // 2 - 1)
            # evacuate combo accumulators so phase A can reuse the banks
            for p in range(3):
                nc.vector.tensor_copy(accb_sb[:, 256 * p:256 * p + 256],
                                      accB[p][:, 0:256])
        nc.sync.dma_start(acc_out[MSLOT:MSLOT + BSLOT, :],
                          accb_sb[0:BSLOT, :])
        nc.sync.dma_start(red_out, red_sb[:])

        # ---- Phase A: far field ----
        with tc.tile_pool(name="pco", bufs=1, space="PSUM") as pco, \
                tc.tile_pool(name="pacct", bufs=1, space="PSUM") as pacct, \
                tc.tile_pool(name="pexp", bufs=2) as pexp:
            accT = [pacct.tile([P, 512], f32, tag=f"at{p}", name=f"at{p}")
                    for p in range(3)]

            def acct_mms(kp):
                for p in range(3):
                    nc.tensor.matmul(
                        accT[p][:, 0:256],
                        wt_sb[:, kp, :, :],
                        stf[:, 2 * kp:2 * kp + 2, 256 * p:256 * p + 256],
                        start=(kp == 0), stop=(kp == WKT // 2 - 1),
                        perf_mode=DR)

            exp_tiles = {}
            co_state = {}

            def co_evac(p, g, on_act):
                if on_act:
                    nc.scalar.copy(co_sb[:, g, :], co_state[p][0][:])
                else:
                    nc.vector.tensor_copy(co_sb[:, g, :], co_state[p][0][:])
                nc.sync.dma_start(co_out[g:g + 1, :], co_sb[0:1, g, :])

            def co_chunks(p, k1, on_act=False):
                cp, k = co_state.get(p, (None, 0))
                co_state[p] = [cp, k]
                while k < k1:
                    if k % 2 == 0:
                        if cp is not None:
                            co_evac(p, CO_GENBASE[p] + k // 2 - 1, on_act)
                        cp = pco.tile([P, 512], f32, tag="co", name="co")
                        co_state[p][0] = cp
                    fo = 256 * (k % 2)
                    nc.tensor.matmul(cp[:, fo:fo + 256], on_sb[:, :, :],
                                     exp_tiles[p][:, 0:2,
                                                  A_CO[p][k]:A_CO[p][k] + 256],
                                     start=True, stop=True, perf_mode=DR)
                    k += 1
                    co_state[p][1] = k

            def co_finish(p, on_act=False):
                k = co_state[p][1]
                co_evac(p, CO_GENBASE[p] + (k - 1) // 2, on_act)

            def fill(p, b, fi):
                qcol = 768 + (2 * p + b) * P
                mms, width, eoff = A_FILLS[p][fi]
                ps = psfill.tile([P, 1024], f32, tag="ps", name="ps")
                o = 0
                for (c0, c1) in mms:
                    nc.tensor.matmul(ps[:, o:o + (c1 - c0)],
                                     xnt_sb[:, qcol:qcol + P],
                                     xnt_sb[:, c0:c1],
                                     start=True, stop=True)
                    o += c1 - c0
                gi = FILL_BASE[2 * p + b] + fi
                nc.scalar.activation(
                    exp_tiles[p][:, b, eoff:eoff + width],
                    ps[:, 0:width], EXP, scale=scale_t,
                    accum_out=rs_sb[:, gi:gi + 1])

            # fill-major schedule, units ordered (1, 2, 0); (unit, fi, b) ->
            # extra work issued right after that fill's sims+exp
            sq_sched = {(1, 0, 1): [12, 13], (1, 1, 0): [14],
                        (1, 1, 1): [15]}
            acct_sched = {(2, 0, 0): [0, 1], (2, 0, 1): [2, 3],
                          (2, 1, 0): [4, 5], (2, 1, 1): [6, 7]}
            UNIT_ORDER = (1, 2, 0)
            pend = None          # (unit, wave-count) colsum work deferred
            for p in UNIT_ORDER:
                exp_tiles[p] = pexp.tile([P, 2, 2304], f8, tag="expu",
                                         name=f"expu{p}")
                for fi in range(len(A_FILLS[p])):
                    for b in range(2):
                        fill(p, b, fi)
                        if pend is not None:
                            co_chunks(*pend)
                            pend = None
                        for kt in sq_sched.get((p, fi, b), []):
                            nc.vector.tensor_mul(stf[:, kt, :],
                                                 sbf[:, kt, :],
                                                 sbf[:, kt, :])
                        for kp in acct_sched.get((p, fi, b), []):
                            acct_mms(kp)
                    pend = (p, A_CO_WAVE[p][fi])
                if p != UNIT_ORDER[-1]:
                    # flush this unit's colsums before moving on
                    if pend is not None:
                        co_chunks(*pend)
                        pend = None
                    co_finish(p)
                if p == 2:
                    # acc_t chains closed; evacuate early (off critical tail)
                    for q_ in range(3):
                        nc.vector.tensor_copy(
                            acct_sb[:, 256 * q_:256 * q_ + 256],
                            accT[q_][:, 0:256])
                    nc.sync.dma_start(acc_out[0:MSLOT, :],
                                      acct_sb[0:MSLOT, :])
            co_chunks(0, A_CO_WAVE[0][-1], on_act=True)
            co_finish(0, on_act=True)

        # ---- write outputs ----
        nc.sync.dma_start(rs_out, rs_sb[:])

    nc.compile()
    return nc


def _get_compiled(scale_t, scale_b):
    key = (round(scale_t, 9), round(scale_b, 9))
    if key not in _compile_cache:
        _compile_cache[key] = _build(scale_t, scale_b)
    return _compile_cache[key]


def _bf16(x):
    b = np.ascontiguousarray(x, dtype=np.float32).view(np.uint32)
    r = ((b.astype(np.uint64) + 0x7FFF + ((b >> 16) & 1)) >> 16 << 16)
    return r.astype(np.uint32).view(np.float32)


def _f8(x):
    import ml_dtypes
    return np.asarray(x, np.float32).astype(
        ml_dtypes.float8_e4m3fn).astype(np.float64)


def _numpy_reference(xn, scale_t, scale_b, tg, bt, wt_w, wb_w):
    """Exact host fallback for label distributions the device layout
    cannot handle (never taken for typical inputs)."""
    f = np.float64
    sim = xn.astype(f) @ xn.astype(f).T
    same_t = tg[:, None] == tg[None, :]
    S_t = np.exp(scale_t * sim)
    diag = np.diagonal(S_t)
    pos = (S_t * same_t).sum(1) - diag
    neg = (S_t * ~same_t).sum(1)
    cnt_pos = same_t.sum(1)
    cnt_neg = (~same_t).sum(1)
    valid = (cnt_pos >= 2) & (cnt_neg >= 1)
    pos_s = np.where(valid, pos, 1.0)
    neg_s = np.where(valid, neg, 1.0)
    loss_i = -np.log(pos_s / (pos_s + neg_s))
    lsum = np.bincount(tg, weights=np.where(valid, loss_i, 0.0),
                       minlength=NT)
    vcnt = np.bincount(tg, weights=valid.astype(f), minlength=NT)
    mean = lsum / np.maximum(vcnt, 1.0)
    lt = np.where(vcnt > 0, mean * np.asarray(wt_w, f), 0.0).sum()
    S_b = np.exp(scale_b * sim)
    same_b = bt[:, None] == bt[None, :]
    pm = same_t & same_b
    nm = same_t & ~same_b
    diag_b = np.diagonal(S_b)
    pos_b = (S_b * pm).sum(1) - diag_b
    neg_b = (S_b * nm).sum(1)
    cpb = pm.sum(1)
    cnb = nm.sum(1)
    valid_b = (cpb >= 2) & (cnb >= 1)
    pos_bs = np.where(valid_b, pos_b, 1.0)
    neg_bs = np.where(valid_b, neg_b, 1.0)
    loss_bi = -np.log(pos_bs / (pos_bs + neg_bs))
    inv = np.where(valid_b, 1.0 / np.where(valid_b, loss_bi, 1.0), 0.0)
    lsum_b = np.bincount(bt, weights=inv, minlength=NB)
    vcnt_b = np.bincount(bt, weights=valid_b.astype(f), minlength=NB)
    mean_b = lsum_b / np.maximum(vcnt_b, 1.0)
    lb = np.where(vcnt_b > 0, mean_b * np.asarray(wb_w, f), 0.0).sum()
    return np.float32(0.9 * lt + 0.1 * lb)


def _run_with_retry(nc, in_maps, core_ids, attempts=3):
    import time as _time

    from concourse.bass_utils import run_bass_kernel_spmd

    for i in range(attempts):
        try:
            return run_bass_kernel_spmd(nc, in_maps, core_ids)
        except Exception:
            if i == attempts - 1:
                raise
            _time.sleep(90)


def kernel(input, temperature, weight_target, weight_batch0, targets, batch0):
    global LAST_RESULT
    import ml_dtypes
    BF = ml_dtypes.bfloat16
    F8 = ml_dtypes.float8_e4m3fn

    x = np.asarray(input, dtype=np.float32)
    t = float(np.clip(np.float32(temperature), MIN_T, MAX_T))
    scale_t, scale_b = 1.0 / t, 1.0 / TEMP_BATCH

    norms = np.sqrt((x * x).sum(axis=1, keepdims=True, dtype=np.float32))
    norms = np.maximum(norms, np.float32(EPS))
    xn = _bf16((x / norms).astype(np.float32))
    tg = np.asarray(targets).astype(np.int64)
    bt = np.asarray(batch0).astype(np.int64)
    combo = tg * NB + bt

    order = np.argsort(combo, kind="stable")
    xs = np.ascontiguousarray(xn[order])
    tgs, bts, cbs = tg[order], bt[order], combo[order]
    s_ii = (xs * xs).sum(axis=1, dtype=np.float32)

    tg_change = np.r_[True, tgs[1:] != tgs[:-1]]
    starts = np.where(tg_change)[0]
    run_id = np.cumsum(tg_change) - 1
    run_ends = np.r_[starts[1:], N]
    cls_start = starts[run_id]
    cls_end = run_ends[run_id]

    # square trick requires scale_t == 2*scale_b
    feasible = abs(scale_t - 2.0 * scale_b) < 1e-9
    slot_t = []           # per-core dict class -> slot
    slot_b = []           # per-core dict combo -> slot
    if feasible:
        for c in range(NCORES):
            # window covers global cols [768c-768, 768c+1280) circularly
            if cls_start[c * R] < 768 * c - 768 or \
                    cls_end[(c + 1) * R - 1] > 768 * c + 1280:
                feasible = False
                break
            rows = slice(c * R, (c + 1) * R)
            ucls = np.unique(tgs[rows])
            if len(ucls) > MSLOT - 1:
                feasible = False
                break
            slot_t.append({int(u): i for i, u in enumerate(ucls)})
            slot_b.append({int(u) * NB + b: i * NB + b
                           for i, u in enumerate(ucls) for b in range(NB)})
    if not feasible:
        return _numpy_reference(xn, scale_t, scale_b, tg, bt,
                                weight_target, weight_batch0)

    xsT = xs.T
    packed_cols = np.r_[0:4608, 5888:6144]
    in_maps = []
    for c in range(NCORES):
        rot = (np.arange(N) + 768 * c - 768) % N
        xnt_full = xsT[:, rot]
        xnt_c = np.ascontiguousarray(xnt_full[:, packed_cols]).astype(BF)
        wkeys = rot[:WKT * P]
        ktg, kcb = tgs[wkeys], cbs[wkeys]
        wt_c = np.zeros((WKT, P, WPAD), np.float32)
        wb_c = np.zeros((WKT, P, WPAD), np.float32)
        wt_c[:, :, MSLOT - 1] = 1.0                      # ones slot
        for cls, sl in slot_t[c].items():
            m = (ktg == cls).reshape(WKT, P)
            wt_c[:, :, sl][m] = 1.0
        for cmb, sl in slot_b[c].items():
            m = (kcb == cmb).reshape(WKT, P)
            wb_c[:, :, sl][m] = 1.0
        in_maps.append({
            "xnt": xnt_c,
            "wt": np.ascontiguousarray(
                wt_c.transpose(1, 0, 2).reshape(P, WKT * WPAD)).astype(F8),
            "wb": np.ascontiguousarray(
                wb_c.transpose(1, 0, 2).reshape(P, WKT * WPAD)).astype(F8),
            "on2": np.ones((P, 256), np.float32).astype(F8),
        })

    nc = _get_compiled(scale_t, scale_b)
    LAST_RESULT = _run_with_retry(nc, in_maps, list(range(NCORES)))

    f = np.float64
    samet2 = np.empty(N); pos4 = np.empty(N); own2 = np.empty(N)
    rowsum4 = np.zeros(N)
    winsum = np.empty(N)
    for c in range(NCORES):
        res = LAST_RESULT.results[c]
        rs = res["rs_out"].astype(f)          # [128, 14]
        am = res["acc_out"].astype(f)         # [48, 768]
        co = res["co_out"].astype(f)          # [12, 512]
        red = res["red_out"].astype(f)        # [128, 4]
        rows = np.arange(c * R, (c + 1) * R)
        lcls = tgs[rows]
        lcmb = cbs[rows]
        st_map = np.array([slot_t[c].get(int(u), 0) for u in range(NT)])
        sb_map = np.array([slot_b[c].get(int(u), 0)
                           for u in range(NT * NB)])
        qidx = np.arange(R)
        pos4[rows] = am[st_map[lcls], qidx]
        winsum[rows] = am[MSLOT - 1, qidx]
        accB = am[MSLOT:]
        own2[rows] = accB[sb_map[lcmb], qidx]
        sam = np.zeros(R)
        for cls, sl in slot_t[c].items():
            sel = lcls == cls
            csl = [slot_b[c][cls * NB + b] for b in range(NB)]
            sam[sel] = accB[csl][:, qidx[sel]].sum(axis=0)
        samet2[rows] = sam
        # own-row partial rowsums (3,3,2,2,2,2 fills per block)
        for bi in range(6):
            f0 = FILL_BASE[bi]
            f1 = FILL_BASE[bi + 1] if bi < 5 else NFILL
            rowsum4[c * R + bi * P:c * R + (bi + 1) * P] += \
                rs[:, f0:f1].sum(axis=1)
        # colsum contributions land on the *key* rows (mirror coverage)
        for p in range(3):
            for k, off in enumerate(A_CO[p]):
                g = CO_GENBASE[p] + k // 2
                h = k % 2
                col0 = 5888 if (p == 0 and off == 2048) \
                    else CO_COLBASE[p] + off
                idx = (col0 + 768 * c - 768 + np.arange(256)) % N
                rowsum4[idx] += co[g, 256 * h:256 * h + 256]
        # window-mirror reduces: rows of window blocks 2..5 gain the keys
        # of this core's p2 unit
        for i, L in enumerate((2, 3, 4, 5)):
            idx = (L * P + 768 * c - 768 + np.arange(P)) % N
            rowsum4[idx] += red[:, i]
    rowsum4 += winsum

    # diagonal terms with exact f8 replication of the device chain
    e2 = _f8(np.exp(scale_b * s_ii.astype(f)))
    e4 = _f8(e2 * e2)

    cnt_t = np.bincount(tgs, minlength=NT)
    n_tb = np.zeros((NT, NB), dtype=np.int64)
    np.add.at(n_tb, (tgs, bts), 1)

    pos_t = pos4 - e4
    neg_t = rowsum4 - pos4
    cnt_pos = cnt_t[tgs]
    cnt_neg = N - cnt_pos
    valid = (cnt_pos >= 2) & (cnt_neg >= 1)
    pos_s = np.where(valid, pos_t, 1.0)
    neg_s = np.where(valid, neg_t, 1.0)
    loss_i = -np.log(pos_s / (pos_s + neg_s))
    lsum = np.bincount(tgs, weights=np.where(valid, loss_i, 0.0),
                       minlength=NT)
    vcnt = np.bincount(tgs, weights=valid.astype(f), minlength=NT)
    mean = lsum / np.maximum(vcnt, 1.0)
    wt_w = np.asarray(weight_target).astype(f)
    loss_target = np.where(vcnt > 0, mean * wt_w, 0.0).sum()

    pos_b = own2 - e2
    neg_b = samet2 - own2
    cnt_pos_b = n_tb[tgs, bts]
    cnt_neg_b = cnt_t[tgs] - cnt_pos_b
    valid_b = (cnt_pos_b >= 2) & (cnt_neg_b >= 1)
    pos_bs = np.where(valid_b, pos_b, 1.0)
    neg_bs = np.where(valid_b, neg_b, 1.0)
    loss_bi = -np.log(pos_bs / (pos_bs + neg_bs))
    inv = np.where(valid_b, 1.0 / np.where(valid_b, loss_bi, 1.0), 0.0)
    lsum_b = np.bincount(bts, weights=inv, minlength=NB)
    vcnt_b = np.bincount(bts, weights=valid_b.astype(f), minlength=NB)
    mean_b = lsum_b / np.maximum(vcnt_b, 1.0)
    wb_w = np.asarray(weight_batch0).astype(f)
    loss_batch = np.where(vcnt_b > 0, mean_b * wb_w, 0.0).sum()

    return np.float32(0.9 * loss_target + 0.1 * loss_batch)
